# revision 1
# baseline (speedup 1.0000x reference)
"""Trainium2 Bass kernel for nn_CustomMultiheadAttention_88158498718125.

Data-parallel over batch: 8 cores x 2 batches each. Full inputs in,
full outputs out; sharding is internal.

Math (per batch-group bg, qc=32, vc=64, h=w=64):
  hw gates   = sigmoid(conv1_w @ [mean_x q; mean_y q] + b1)
  diag gate  = sigmoid(diag_w @ mean_i(k[i,i]+k[i,63-i]) + db)
  x1_pre     = v * (h_gate*w_gate + diag_gate)
  GroupNorm stats of x1_pre -> a1 = softmax_c(normalized per-channel means)
  att        = conv3x3(v) contracted with a1  == single-channel conv with
               a1-mixed weights w_eff  (+ a1 . conv3_b)
  out        = out_w @ (v * sigmoid(att)) + out_b

The normalized x1 tensor is never materialized (only its moments are
needed), and the 64->64-channel conv3x3 is never computed (only its
a1-contraction via 9 shifted 8-row matmuls), which removes almost all
of the reference FLOPs. Bulk data is bf16 (PE full rate + DVE 2x
modes); statistics accumulate in fp32 and cross-partition reductions
go through gpsimd.partition_all_reduce.
"""

import os
import sys

sys.path.insert(0, "/opt/trn_rl_repo")

KPHASES = int(os.environ.get("KPHASES", "9"))

import numpy as np

import concourse.bass as bass
import concourse.bacc as bacc
import concourse.mybir as mybir
import concourse.tile as tile
from concourse.masks import make_identity

F32 = mybir.dt.float32
BF16 = mybir.dt.bfloat16
AF = mybir.ActivationFunctionType
ALU = mybir.AluOpType
AX = mybir.AxisListType

B = 16          # full batch
B_LOC = 2       # batches per core
N_CORES = 8
G = 8           # groups
QD, VD = 256, 512
QC, VC = QD // G, VD // G   # 32, 64
H = W = 64
S = H * W                   # 4096
PW = W + 2                  # padded row stride 66
PS = PW * PW                # 4356
NCH = 8                     # 512-wide spatial chunks
CH = S // NCH               # 512
EPS = 1e-5

PARAM_NAMES = [
    "conv1_w", "conv1_b", "conv3_w", "conv3_b",
    "gn_w", "gn_b", "diag_w", "diag_b", "out_w", "out_b",
]


def build_program():
    nc = bacc.Bacc("TRN2", target_bir_lowering=False)

    q_d = nc.declare_dram_parameter("q", [B_LOC, QD, H, W], F32, isOutput=False)
    k_d = nc.declare_dram_parameter("k", [B_LOC, QD, H, W], F32, isOutput=False)
    v_d = nc.declare_dram_parameter("v", [B_LOC, VD, H, W], F32, isOutput=False)
    c1w_d = nc.declare_dram_parameter("conv1_w", [VC, QC], F32, isOutput=False)
    c1b_d = nc.declare_dram_parameter("conv1_b", [VC], F32, isOutput=False)
    c3w_d = nc.declare_dram_parameter("conv3_w", [VC, VC, 3, 3], F32, isOutput=False)
    c3b_d = nc.declare_dram_parameter("conv3_b", [VC], F32, isOutput=False)
    gnw_d = nc.declare_dram_parameter("gn_w", [VC], F32, isOutput=False)
    gnb_d = nc.declare_dram_parameter("gn_b", [VC], F32, isOutput=False)
    dww_d = nc.declare_dram_parameter("diag_w", [VC, QC], F32, isOutput=False)
    dwb_d = nc.declare_dram_parameter("diag_b", [VC], F32, isOutput=False)
    oww_d = nc.declare_dram_parameter("out_w", [VD, VD], F32, isOutput=False)
    owb_d = nc.declare_dram_parameter("out_b", [VD], F32, isOutput=False)
    out_d = nc.declare_dram_parameter("out", [B_LOC, VD, H, W], F32, isOutput=True)

    with tile.TileContext(nc) as tc:
        from contextlib import ExitStack

        with ExitStack() as ctx:
            _body(ctx, tc, nc, q_d, k_d, v_d, c1w_d, c1b_d, c3w_d, c3b_d,
                  gnw_d, gnb_d, dww_d, dwb_d, oww_d, owb_d, out_d)
    nc.finalize()
    return nc


def _body(ctx, tc, nc, q_d, k_d, v_d, c1w_d, c1b_d, c3w_d, c3b_d,
          gnw_d, gnb_d, dww_d, dwb_d, oww_d, owb_d, out_d):
    const = ctx.enter_context(tc.tile_pool(name="const", bufs=1))
    qpool = ctx.enter_context(tc.tile_pool(name="qt", bufs=2))
    v1pool = ctx.enter_context(tc.tile_pool(name="v1ch", bufs=3))
    vbfpool = ctx.enter_context(tc.tile_pool(name="vbf", bufs=5))
    mpool = ctx.enter_context(tc.tile_pool(name="mch", bufs=3))
    tpool = ctx.enter_context(tc.tile_pool(name="tch", bufs=3))
    gates = ctx.enter_context(tc.tile_pool(name="gates", bufs=10))
    small = ctx.enter_context(tc.tile_pool(name="small", bufs=2))
    zlpool = ctx.enter_context(tc.tile_pool(name="zl", bufs=5))
    zsbpool = ctx.enter_context(tc.tile_pool(name="zsb", bufs=3))
    zbpool = ctx.enter_context(tc.tile_pool(name="zbig", bufs=2))
    sigpool = ctx.enter_context(tc.tile_pool(name="sig", bufs=2))
    gchpool = ctx.enter_context(tc.tile_pool(name="gch", bufs=6))
    osbpool = ctx.enter_context(tc.tile_pool(name="osb", bufs=4))

    ps_proj = ctx.enter_context(tc.tile_pool(name="psproj", bufs=4, space="PSUM"))
    ps_za = ctx.enter_context(tc.tile_pool(name="psza", bufs=2, space="PSUM"))
    ps_small = ctx.enter_context(tc.tile_pool(name="pssmall", bufs=1, space="PSUM"))

    # ---------------- constants / weights staging ----------------
    id128bf = const.tile([128, 128], BF16)
    make_identity(nc, id128bf[:])

    # out_w transposed (bf16): owT[:, 512*cc + o] = out_w[o, 128*cc + p]
    owT = const.tile([128, 2048], BF16)
    for oc in range(4):
        ow_t = qpool.tile([128, 512], F32, tag="owld", bufs=2)
        nc.sync.dma_start(ow_t[:], oww_d[128 * oc:128 * (oc + 1), :])
        ow_tb = qpool.tile([128, 512], BF16, tag="owldb", bufs=2)
        nc.vector.tensor_copy(ow_tb[:], ow_t[:])
        for cc in range(4):
            tps = ps_small.tile([128, 128], BF16, tag="spsb")
            nc.tensor.transpose(tps[:], ow_tb[:, 128 * cc:128 * (cc + 1)], id128bf[:])
            nc.vector.tensor_copy(owT[:, 512 * cc + 128 * oc:512 * cc + 128 * oc + 128], tps[:])

    # conv1 / diag_w transposed block-diagonal lhsT tiles (bf16)
    c1blkT = []
    dwblkT = []
    for mg in range(2):
        c1r = qpool.tile([128, 128], F32, name=f"c1r{mg}", tag="blkraw", bufs=2)
        dwr = qpool.tile([128, 128], F32, name=f"dwr{mg}", tag="blkraw2", bufs=2)
        nc.vector.memset(c1r[:], 0.0)
        nc.vector.memset(dwr[:], 0.0)
        base = 64 * mg
        for gi in range(2):
            nc.sync.dma_start(
                c1r[base + 32 * gi:base + 32 * gi + 32, 64 * gi:64 * gi + 64],
                c1w_d[:].rearrange("o c -> c o"))
            nc.sync.dma_start(
                dwr[base + 32 * gi:base + 32 * gi + 32, 64 * gi:64 * gi + 64],
                dww_d[:].rearrange("o c -> c o"))
        c1 = const.tile([128, 128], BF16, name=f"c1blk{mg}", tag=f"c1blk{mg}")
        dw = const.tile([128, 128], BF16, name=f"dwblk{mg}", tag=f"dwblk{mg}")
        nc.vector.tensor_copy(c1[:], c1r[:])
        nc.vector.tensor_copy(dw[:], dwr[:])
        c1blkT.append(c1)
        dwblkT.append(dw)

    # W3 tap-major (bf16): W3b[par*64+c, 64*t + ci] = conv3_w[c, ci, t];
    # col 576 = conv3_b
    W3r = qpool.tile([128, 577], F32, tag="w3raw", bufs=1)
    c3v = c3w_d[:].rearrange("c ci dy dx -> c ci (dy dx)")
    for par in range(2):
        for t9 in range(9):
            nc.sync.dma_start(
                W3r[64 * par:64 * par + 64, 64 * t9:64 * t9 + 64],
                c3v[:, :, t9])
        nc.sync.dma_start(W3r[64 * par:64 * par + 64, 576:577], c3b_d[:].unsqueeze(1))
    W3b = const.tile([128, 577], BF16)
    nc.vector.tensor_copy(W3b[:], W3r[:])

    # per-partition param vectors duplicated across parities
    def dup2(src, nm):
        t = const.tile([128, 1], F32, name=nm, tag=nm)
        for par in range(2):
            nc.sync.dma_start(t[64 * par:64 * par + 64, :], src[:].unsqueeze(1))
        return t

    b1t = dup2(c1b_d, "b1t")
    dbt = dup2(dwb_d, "dbt")
    gnw2 = dup2(gnw_d, "gnw2")
    gnb2 = dup2(gnb_d, "gnb2")

    ob4 = const.tile([128, 4], F32)
    for oc in range(4):
        nc.sync.dma_start(ob4[:, oc:oc + 1], owb_d[128 * oc:128 * (oc + 1)].unsqueeze(1))

    epst = const.tile([128, 1], F32)
    nc.vector.memset(epst[:], EPS)

    # ones2b[p, j] = 1 if p//64 == j (parity-sum matmul lhsT, bf16)
    ones2b = const.tile([128, 2], BF16)
    nc.vector.memset(ones2b[:], 0.0)
    nc.vector.memset(ones2b[0:64, 0:1], 1.0)
    nc.vector.memset(ones2b[64:128, 1:2], 1.0)
    # indicator [2,128]: row p broadcast to its 64-partition half = ones2b^T
    ind2b = const.tile([2, 128], BF16)
    i2ps = ps_small.tile([128, 128], BF16, tag="spsb")
    nc.tensor.transpose(i2ps[0:2, 0:128], ones2b[:], id128bf[:])
    nc.vector.tensor_copy(ind2b[:], i2ps[0:2, 0:128])

    # ind8[j][row, m] = 1 if row == 2j + (m >= 64): broadcasts sig rows
    # (2j, 2j+1) to the 128 partitions of v-pack j via a K=8 matmul (bf16).
    ind8 = []
    for j in range(4):
        cp = const.tile([128, 8], BF16, name=f"cp8_{j}", tag=f"cp8_{j}")
        nc.vector.memset(cp[:], 0.0)
        nc.vector.memset(cp[0:64, 2 * j:2 * j + 1], 1.0)
        nc.vector.memset(cp[64:128, 2 * j + 1:2 * j + 2], 1.0)
        t8 = const.tile([8, 128], BF16, name=f"ind8_{j}", tag=f"ind8_{j}")
        t8ps = ps_small.tile([128, 128], BF16, tag="spsb")
        nc.tensor.transpose(t8ps[0:8, 0:128], cp[:], id128bf[:])
        nc.vector.tensor_copy(t8[:], t8ps[0:8, 0:128])
        ind8.append(t8)

    # sel72[8t + i, i] = 1: sums the 9 pre-shifted tap rows per bg in one
    # K=72 matmul (all operands at partition base 0).
    sel72 = const.tile([72, 8], BF16)
    nc.vector.memset(sel72[:], 0.0)
    for t9 in range(9):
        nc.sync.dma_start(sel72[8 * t9:8 * t9 + 8, 0:8], id128bf[0:8, 0:8])

    for b in range(B_LOC):
        # z scatter target: row 8t + bgb holds tap t of batch-group bgb,
        # written pre-shifted so all taps read at one uniform offset.
        zbig = zbpool.tile([128, PS], BF16, tag="zbig")

        # ---------------- Q phase: coordinate gates ----------------
        hwg = []   # per v-pack [128,128] bf16: cols 0:64 h-gate(y), 64:128 w-gate(x)
        gd = []    # per v-pack [128,1] bf16: diag gate
        for jp in range(2):
            qt = qpool.tile([128, S], F32, tag="qt")
            nc.sync.dma_start(qt[:], q_d[b, 128 * jp:128 * (jp + 1)].rearrange("c h w -> c (h w)"))
            hwf = small.tile([128, 128], BF16, tag="hwf")
            with nc.allow_low_precision(reason="bf16 gate logits, fp32 internal accum"):
                nc.vector.reduce_sum(hwf[:, 0:64], qt[:].rearrange("c (h w) -> c h w", h=H),
                                     axis=AX.X)
                nc.vector.reduce_sum(hwf[:, 64:128], qt[:].rearrange("c (h w) -> c w h", h=H),
                                     axis=AX.X)

            # k diagonals
            ksl = k_d[b, 128 * jp:128 * (jp + 1)].rearrange("c h w -> c (h w)")
            kd = small.tile([128, 128], F32, tag="kd")
            nc.sync.dma_start(kd[:, 0:64], ksl[:, 0:4096:65])
            nc.sync.dma_start(kd[:, 64:128], ksl[:, 63:4095:63])
            kds = small.tile([128, 1], BF16, tag="kds")
            with nc.allow_low_precision(reason="bf16 gate logits, fp32 internal accum"):
                nc.vector.reduce_sum(kds[:], kd[:], axis=AX.X)

            for mg in range(2):
                hw_ps = ps_small.tile([128, 128], F32, tag="sps")
                nc.tensor.matmul(hw_ps[:], c1blkT[mg][:], hwf[:], start=True, stop=True)
                hg = gates.tile([128, 128], BF16, tag="hwg")
                nc.scalar.activation(hg[:], hw_ps[:], AF.Sigmoid,
                                     bias=b1t[:], scale=1.0 / 64)
                hwg.append(hg)

                g_ps = ps_small.tile([128, 128], F32, tag="sps")
                nc.tensor.matmul(g_ps[0:128, 0:1], dwblkT[mg][:], kds[:],
                                 start=True, stop=True)
                gt = gates.tile([128, 1], BF16, tag="gd")
                nc.scalar.activation(gt[:], g_ps[0:128, 0:1], AF.Sigmoid,
                                     bias=dbt[:], scale=1.0 / 64)
                gd.append(gt)

        if KPHASES < 2:
            continue
        # ------ stats + a1 phase (per pack), with bf16 staging of v ------
        vbf = []
        weffL = small.tile([128, 8], BF16, tag="weffL")
        nc.vector.memset(weffL[:], 0.0)
        for j in range(4):
            vsl = v_d[b, 128 * j:128 * (j + 1)].rearrange("c h w -> c (h w)")
            vb = vbfpool.tile([128, S], BF16, tag="vbf")
            s1p = small.tile([128, NCH], F32, tag="s1p")
            s2p = small.tile([128, NCH], F32, tag="s2p")
            for ch in range(NCH):
                sl = slice(CH * ch, CH * (ch + 1))
                SL = int(os.environ.get("STATS_LVL", "5"))
                v1 = v1pool.tile([128, CH], F32)
                nc.scalar.dma_start(v1[:], vsl[:, sl])
                nc.vector.tensor_copy(vb[:, sl], v1[:])
                if SL < 2:
                    nc.vector.memset(s1p[:, ch:ch + 1], 0.0)
                    nc.vector.memset(s2p[:, ch:ch + 1], 0.0)
                    continue
                mch = mpool.tile([128, CH], BF16)
                hv = hwg[j][:, 8 * ch:8 * (ch + 1)].unsqueeze(2).broadcast_to([128, 8, 64])
                wv = hwg[j][:, 64:128].unsqueeze(1).broadcast_to([128, 8, 64])
                m3 = mch[:].rearrange("c (h w) -> c h w", h=8)
                if os.environ.get("M_DVE", "0") == "1":
                    nc.vector.tensor_tensor(m3, hv, wv, op=ALU.mult)
                else:
                    nc.gpsimd.tensor_tensor(m3, hv, wv, op=ALU.mult)
                if SL < 3:
                    nc.vector.memset(s1p[:, ch:ch + 1], 0.0)
                    nc.vector.memset(s2p[:, ch:ch + 1], 0.0)
                    continue
                tch = tpool.tile([128, CH], BF16)
                if SL < 4:
                    nc.vector.scalar_tensor_tensor(tch[:], mch[:], gd[j][:], vb[:, sl],
                                                   op0=ALU.add, op1=ALU.mult)
                    nc.vector.reduce_sum(s1p[:, ch:ch + 1], tch[:], axis=AX.X)
                    nc.vector.memset(s2p[:, ch:ch + 1], 0.0)
                    continue
                nc.vector.scalar_tensor_tensor(tch[:], mch[:], gd[j][:], vb[:, sl],
                                               op0=ALU.add, op1=ALU.mult,
                                               accum_out=s1p[:, ch:ch + 1])
                if SL < 5:
                    nc.vector.memset(s2p[:, ch:ch + 1], 0.0)
                    continue
                sq = tpool.tile([128, CH], BF16, tag="sq")
                nc.vector.scalar_tensor_tensor(sq[:], tch[:], 1.0, tch[:],
                                               op0=ALU.mult, op1=ALU.mult,
                                               accum_out=s2p[:, ch:ch + 1])
            vbf.append(vb)

            # per-partition mean / E[x^2], then per-bg (parity) group stats
            statp = small.tile([128, 2], BF16, tag="statp")
            pmu = small.tile([128, 1], F32, tag="pmu")
            s1s = small.tile([128, 1], F32, tag="s1s")
            nc.vector.reduce_sum(s1s[:], s1p[:], axis=AX.X)
            nc.vector.tensor_scalar_mul(pmu[:], s1s[:], 1.0 / S)
            nc.vector.tensor_copy(statp[:, 0:1], pmu[:])
            s2s = small.tile([128, 1], F32, tag="s2s")
            nc.vector.reduce_sum(s2s[:], s2p[:], axis=AX.X)
            nc.vector.tensor_scalar_mul(statp[:, 1:2], s2s[:], 1.0 / S)

            stat_ps = ps_small.tile([2, 8], F32, tag="sps")
            nc.tensor.matmul(stat_ps[0:2, 0:2], ones2b[:], statp[:],
                             start=True, stop=True)
            stats_sb = small.tile([2, 2], BF16, tag="statsb")
            nc.vector.tensor_copy(stats_sb[:], stat_ps[0:2, 0:2])
            bc_ps = ps_small.tile([128, 128], F32, tag="sps")
            nc.tensor.matmul(bc_ps[0:128, 0:2], ind2b[:], stats_sb[:],
                             start=True, stop=True)
            mu = small.tile([128, 1], F32, tag="mu")
            nc.vector.tensor_scalar_mul(mu[:], bc_ps[0:128, 0:1], 1.0 / 64)
            ex2 = small.tile([128, 1], F32, tag="ex2")
            nc.vector.tensor_scalar_mul(ex2[:], bc_ps[0:128, 1:2], 1.0 / 64)
            negvar = small.tile([128, 1], F32, tag="negvar")
            nc.vector.scalar_tensor_tensor(negvar[:], mu[:], mu[:], ex2[:],
                                           op0=ALU.mult, op1=ALU.subtract)
            stdv = small.tile([128, 1], F32, tag="stdv")
            nc.scalar.activation(stdv[:], negvar[:], AF.Sqrt, bias=epst[:], scale=-1.0)
            inv = small.tile([128, 1], F32, tag="inv")
            nc.vector.reciprocal(inv[:], stdv[:])
            dmu = small.tile([128, 1], F32, tag="dmu")
            nc.vector.tensor_sub(dmu[:], pmu[:], mu[:])
            sca = small.tile([128, 1], F32, tag="sca")
            nc.vector.tensor_mul(sca[:], inv[:], gnw2[:])
            logit = small.tile([128, 1], F32, tag="logit")
            nc.vector.scalar_tensor_tensor(logit[:], dmu[:], sca[:], gnb2[:],
                                           op0=ALU.mult, op1=ALU.add)
            etb = small.tile([128, 1], BF16, tag="etb")
            nc.scalar.activation(etb[:], logit[:], AF.Exp)
            den_ps = ps_small.tile([2, 8], F32, tag="sps")
            nc.tensor.matmul(den_ps[0:2, 0:1], ones2b[:], etb[:], start=True, stop=True)
            den_sb = small.tile([2, 1], F32, tag="densb")
            nc.vector.tensor_copy(den_sb[:], den_ps[0:2, 0:1])
            rden = small.tile([2, 1], F32, tag="rden")
            nc.vector.reciprocal(rden[:], den_sb[:])
            rdenb = small.tile([2, 1], BF16, tag="rdenb")
            nc.vector.tensor_copy(rdenb[:], rden[:])
            rd_ps = ps_small.tile([128, 128], F32, tag="sps")
            nc.tensor.matmul(rd_ps[0:128, 0:1], ind2b[:], rdenb[:], start=True, stop=True)
            # a1 (softmax over the 64 channels of each bg), written into the
            # block-structured w_eff mixing lhsT
            nc.vector.tensor_mul(weffL[0:64, 2 * j:2 * j + 1],
                                 etb[0:64, :], rd_ps[0:64, 0:1])
            nc.vector.tensor_mul(weffL[64:128, 2 * j + 1:2 * j + 2],
                                 etb[64:128, :], rd_ps[64:128, 0:1])

        if KPHASES < 3:
            continue
        # ---------------- w_eff mixing + z lhsT scatter ----------------
        weff_main = ps_za.tile([8, 512], F32, tag="za")
        nc.tensor.matmul(weff_main[:], weffL[:], W3b[:, 0:512], start=True, stop=True)
        weff_ex = ps_small.tile([8, 65], F32, tag="sps")
        nc.tensor.matmul(weff_ex[:], weffL[:], W3b[:, 512:577], start=True, stop=True)
        wf_sb = small.tile([8, 576], BF16, tag="wfsb")
        nc.vector.tensor_copy(wf_sb[:, 0:512], weff_main[:])
        nc.vector.tensor_copy(wf_sb[:, 512:576], weff_ex[:, 0:64])
        btt = small.tile([8, 1], F32, tag="btt")
        nc.vector.tensor_copy(btt[:], weff_ex[:, 64:65])

        # Transpose w_eff on the PE: 9 small matmuls W3b-slice^T @ weffL give
        # wT[64t+ci (mod 128), bg] in psum; even slices put tap t at rows
        # 64*(t%2), 64-shifted (odd) slices put it at rows 64*((t+1)%2), so
        # every zlj write below is a quadrant-aligned DVE copy (no DMA).
        wT_sb = small.tile([128, 72], BF16, tag="wTsb")
        nchunks = []
        for k in range(5):
            lo = 128 * k
            nchunks.append((lo, min(lo + 128, 576)))
        for k in range(4):
            lo = 64 + 128 * k
            nchunks.append((lo, lo + 128))
        for idx, (lo, hi) in enumerate(nchunks):
            wt_ps = ps_small.tile([128, 8], F32, tag="sps")
            m = hi - lo
            nc.tensor.matmul(wt_ps[0:m, :], W3b[:, lo:hi], weffL[:],
                             start=True, stop=True)
            nc.vector.tensor_copy(wT_sb[0:m, 8 * idx:8 * idx + 8], wt_ps[0:m, :])

        zl = []
        for j in range(4):
            zlj = zlpool.tile([128, 18], BF16)
            nc.vector.memset(zlj[:], 0.0)
            for t9 in range(9):
                # parity 0 rows 0:64
                if t9 % 2 == 0:
                    idx, roff = t9 // 2, 0          # even chunk, rows 0-63
                else:
                    idx, roff = 5 + t9 // 2, 0      # odd chunk, rows 0-63
                nc.vector.tensor_copy(
                    zlj[0:64, 2 * t9:2 * t9 + 1],
                    wT_sb[roff:roff + 64, 8 * idx + 2 * j:8 * idx + 2 * j + 1])
                # parity 1 rows 64:128
                if t9 == 0:
                    nc.sync.dma_start(zlj[64:128, 1:2],
                                      wf_sb[2 * j + 1:2 * j + 2, 0:64])
                    continue
                if t9 % 2 == 1:
                    idx, roff = t9 // 2, 64         # even chunk, rows 64-127
                else:
                    idx, roff = 5 + t9 // 2 - 1, 64  # odd chunk, rows 64-127
                nc.vector.tensor_copy(
                    zlj[64:128, 2 * t9 + 1:2 * t9 + 2],
                    wT_sb[roff:roff + 64, 8 * idx + 2 * j:8 * idx + 2 * j + 1])
            zl.append(zlj)

        if KPHASES < 4:
            continue
        # ---------------- Z phase: z = w_eff . v  (18 tap-rows) ----------------
        nc.vector.memset(zbig[:], 0.0)

        for j in range(4):
            zsb = zsbpool.tile([18, S], BF16)
            for ch in range(NCH):
                z_ps = ps_za.tile([18, 512], F32, tag="za")
                nc.tensor.matmul(z_ps[:], zl[j][:], vbf[j][:, CH * ch:CH * (ch + 1)],
                                 start=True, stop=True)
                nc.scalar.copy(zsb[:, CH * ch:CH * (ch + 1)], z_ps[:])
            for t9 in range(9):
                dy, dx = divmod(t9, 3)
                src = zsb[2 * t9:2 * t9 + 2, :]
                dst = zbig[8 * t9 + 2 * j:8 * t9 + 2 * j + 2].rearrange(
                    "r (yy xx) -> r yy xx", yy=PW)[
                    :, 2 - dy:66 - dy, 2 - dx:66 - dx]
                nc.scalar.dma_start(dst, src.rearrange("t (y x) -> t y x", y=H))

        # All zb scatter DMAs must land before the att matmuls read them --
        # Tile's region tracking under-syncs the multi-queue scatter writes.
        tc.strict_bb_all_engine_barrier()

        if KPHASES < 5:
            continue
        # ---------------- att + gating + projection (chunk-wise) ----------------
        for sc in range(NCH):
            att_ps = ps_za.tile([8, 512], F32, tag="za")
            rhs = zbig[0:72].rearrange(
                "r (yy xx) -> r yy xx", yy=PW)[:, 8 * sc + 1:8 * sc + 9, 1:65]
            nc.tensor.matmul(att_ps[:], sel72[:], rhs, start=True, stop=True)
            sig = sigpool.tile([8, CH], BF16)
            nc.scalar.activation(sig[:], att_ps[:], AF.Sigmoid, bias=btt[:], scale=1.0)
            if int(os.environ.get("ATT_LVL", "3")) < 2:
                continue

            gch = []
            for j in range(4):
                bc_ps = ps_za.tile([128, 512], F32, tag="za")
                nc.tensor.matmul(bc_ps[:], ind8[j][:], sig[:], start=True, stop=True)
                g = gchpool.tile([128, CH], BF16, tag="gch")
                nc.vector.tensor_mul(g[:], vbf[j][:, CH * sc:CH * (sc + 1)], bc_ps[:])
                gch.append(g)

            if int(os.environ.get("ATT_LVL", "3")) < 3:
                continue
            for oc in range(4):
                pps = ps_proj.tile([128, 512], F32, tag="proj")
                for j in range(4):
                    nc.tensor.matmul(pps[:], owT[:, 512 * j + 128 * oc:512 * j + 128 * oc + 128],
                                     gch[j][:], start=(j == 0), stop=(j == 3))
                osb = osbpool.tile([128, CH], F32)
                nc.scalar.activation(osb[:], pps[:], AF.Identity,
                                     bias=ob4[:, oc:oc + 1], scale=1.0)
                nc.sync.dma_start(
                    out_d[b, 128 * oc:128 * (oc + 1)].rearrange("c h w -> c (h w)")[
                        :, CH * sc:CH * (sc + 1)], osb[:])


_NC_CACHE = None


def _get_nc():
    global _NC_CACHE
    if _NC_CACHE is None:
        _NC_CACHE = build_program()
    return _NC_CACHE


def kernel(**inputs):
    from concourse.bass_utils import run_bass_kernel_spmd

    nc = _get_nc()
    q = np.asarray(inputs["q"], np.float32)
    k = np.asarray(inputs["k"], np.float32)
    v = np.asarray(inputs["v"], np.float32)
    params = {n: np.asarray(inputs[n], np.float32) for n in PARAM_NAMES}

    core_ids = list(range(N_CORES))
    in_maps = []
    for c in core_ids:
        sl = slice(B_LOC * c, B_LOC * (c + 1))
        in_maps.append({"q": q[sl], "k": k[sl], "v": v[sl], **params})

    res = run_bass_kernel_spmd(nc, in_maps, core_ids)
    out = np.concatenate([res.results[c]["out"] for c in core_ids], axis=0)
    return out



# revision 6
# speedup vs baseline: 3.0490x; 3.0490x over previous
"""Trainium2 Bass kernel for nn_CustomMultiheadAttention_88158498718125.

Data-parallel over batch: 8 cores x 2 batches each. Full inputs in,
full outputs out; sharding is internal.

Math (per batch-group bg, qc=32, vc=64, h=w=64):
  hw gates   = sigmoid(conv1_w @ [mean_x q; mean_y q] + b1)
  diag gate  = sigmoid(diag_w @ mean_i(k[i,i]+k[i,63-i]) + db)
  x1_pre     = v * (h_gate*w_gate + diag_gate)
  GroupNorm stats of x1_pre -> a1 = softmax_c(normalized per-channel means)
  att        = conv3x3(v) contracted with a1  == per-tap channel mixes
               z_t = weff[:,t].v summed over 9 shifted taps (+ a1 . conv3_b)
  out        = out_w @ (v * sigmoid(att)) + out_b

Key design points vs the naive formulation:
  * x1 is never materialized (only GroupNorm moments), and the 64->64
    conv3x3 never runs: a1 contracts conv3_w into w_eff [ci, 9] first.
  * z lives in a 66x66 zero-padded, pack-major layout (quadrant rows
    32j+2t+b) written by plain engine copies from PSUM; the 9-tap sum
    is 9 accumulating PE matmuls whose rhs reads z at the tap's
    (dy,dx) window offset.  No shifted scatter DMAs at all.
  * All weight-derived staging (out_w transpose, conv3 tap-major pack,
    block-diag conv1/diag lhsT, selection/indicator matrices) is
    precomputed on host and DMA'd in as extra DRAM consts.
  * Bulk data is bf16 (cast during DMA by the Pool SWDGE queue); f32
    only for statistics and the final projection output.
"""

import sys

sys.path.insert(0, "/opt/trn_rl_repo")

import numpy as np

import concourse.bass as bass
import concourse.bacc as bacc
import concourse.mybir as mybir
import concourse.tile as tile

F32 = mybir.dt.float32
BF16 = mybir.dt.bfloat16
AF = mybir.ActivationFunctionType
ALU = mybir.AluOpType
AX = mybir.AxisListType

B = 16          # full batch
B_LOC = 2       # batches per core
N_CORES = 8
G = 8           # groups
QD, VD = 256, 512
QC, VC = QD // G, VD // G   # 32, 64
H = W = 64
S = H * W                   # 4096
PW = W + 2                  # padded row stride 66
PS = PW * PW                # 4356
NCH = 8                     # 512-wide spatial chunks
CH = S // NCH               # 512
EPS = 1e-5

PARAM_NAMES = [
    "conv1_w", "conv1_b", "conv3_w", "conv3_b",
    "gn_w", "gn_b", "diag_w", "diag_b", "out_w", "out_b",
]

# host-precomputed const layout (single f32 DRAM tensor, cast on load)
#   cb  [128, 2048+577+4*128+72] : owT | W3b | c1blk0 c1blk1 dwblk0 dwblk1 | sel
#   cb2 [8, 512+2+128]           : ind8 (4x128) | unused pad
#   cf  [128, 8]                 : b1t dbt gnw2 gnb2 ob4(4) eps
CB_OWT = 0
CB_W3B = 2048
CB_BLK = 2048 + 577
CB_SEL = CB_BLK + 4 * 128
CB_COLS = CB_SEL + 72

C2_IND8 = 0
C2_ONES2 = 512           # ones2 [128,2] does not fit here; see cf2
C2_COLS = 512

CF_B1 = 0
CF_DB = 1
CF_GNW = 2
CF_GNB = 3
CF_OB = 4      # 4 cols
CF_COLS = 8


def _host_consts(params):
    """Build the packed const tensors from the (float32) params."""
    c1w = params["conv1_w"]          # [64, 32]
    c3w = params["conv3_w"]          # [64, 64, 3, 3]
    dww = params["diag_w"]           # [64, 32]
    oww = params["out_w"]            # [512, 512]

    cb = np.zeros((128, CB_COLS), np.float32)
    # owT[p, 512*cc + 128*oc + i] = out_w[128*oc + i, 128*cc + p]
    for cc in range(4):
        for oc in range(4):
            blk = oww[128 * oc:128 * (oc + 1), 128 * cc:128 * (cc + 1)]
            cb[:, CB_OWT + 512 * cc + 128 * oc:CB_OWT + 512 * cc + 128 * oc + 128] = blk.T
    # W3b[64*par + c, 64*t + ci] = conv3_w[c, ci, t]; col 576 = conv3_b
    w3 = c3w.reshape(VC, VC, 9)
    for par in range(2):
        for t9 in range(9):
            cb[64 * par:64 * par + 64, CB_W3B + 64 * t9:CB_W3B + 64 * t9 + 64] = w3[:, :, t9]
        cb[64 * par:64 * par + 64, CB_W3B + 576] = params["conv3_b"]
    # c1blk/dwblk [mg][64*mg + 32*gi + c, 64*gi + o] = w[o, c]
    for which, wsrc in ((0, c1w), (1, dww)):
        for mg in range(2):
            col0 = CB_BLK + (2 * which + mg) * 128
            for gi in range(2):
                cb[64 * mg + 32 * gi:64 * mg + 32 * gi + 32,
                   col0 + 64 * gi:col0 + 64 * gi + 64] = wsrc.T
    # sel[32*j + 2*t + b, 8*t + 2*j + b] = 1  (att tap-selection lhsT)
    for j in range(4):
        for t9 in range(9):
            for bb in range(2):
                cb[32 * j + 2 * t9 + bb, CB_SEL + 8 * t9 + 2 * j + bb] = 1.0

    cb2 = np.zeros((8, C2_COLS), np.float32)
    # ind8[r, 128*j + p] = 1 iff r == 2*j + p//64
    for j in range(4):
        for p in range(128):
            cb2[2 * j + p // 64, C2_IND8 + 128 * j + p] = 1.0

    cf = np.zeros((128, CF_COLS), np.float32)
    cf[:, CF_B1] = np.tile(params["conv1_b"], 2)
    cf[:, CF_DB] = np.tile(params["diag_b"], 2)
    cf[:, CF_GNW] = np.tile(params["gn_w"], 2)
    cf[:, CF_GNB] = np.tile(params["gn_b"], 2)
    for oc in range(4):
        cf[:, CF_OB + oc] = params["out_b"][128 * oc:128 * (oc + 1)]

    # ones2 [128, 2]: parity-sum matmul lhsT; ind2 [2, 128] its transpose
    ones2 = np.zeros((128, 2), np.float32)
    ones2[0:64, 0] = 1.0
    ones2[64:128, 1] = 1.0
    ind2 = ones2.T.copy()

    return cb, cb2, cf, ones2, ind2


def build_program():
    nc = bacc.Bacc("TRN2", target_bir_lowering=False)

    q_d = nc.declare_dram_parameter("q", [B_LOC, QD, H, W], F32, isOutput=False)
    k_d = nc.declare_dram_parameter("k", [B_LOC, QD, H, W], F32, isOutput=False)
    v_d = nc.declare_dram_parameter("v", [B_LOC, VD, H, W], F32, isOutput=False)
    cb_d = nc.declare_dram_parameter("cb", [128, CB_COLS], F32, isOutput=False)
    cb2_d = nc.declare_dram_parameter("cb2", [8, C2_COLS], F32, isOutput=False)
    cf_d = nc.declare_dram_parameter("cf", [128, CF_COLS], F32, isOutput=False)
    ones2_d = nc.declare_dram_parameter("ones2", [128, 2], F32, isOutput=False)
    ind2_d = nc.declare_dram_parameter("ind2", [2, 128], F32, isOutput=False)
    out_d = nc.declare_dram_parameter("out", [B_LOC, VD, H, W], F32, isOutput=True)

    with tile.TileContext(nc) as tc:
        from contextlib import ExitStack

        with ExitStack() as ctx:
            _body(ctx, tc, nc, q_d, k_d, v_d, cb_d, cb2_d, cf_d, ones2_d,
                  ind2_d, out_d)
    nc.finalize()
    return nc


def _body(ctx, tc, nc, q_d, k_d, v_d, cb_d, cb2_d, cf_d, ones2_d, ind2_d,
          out_d):
    const = ctx.enter_context(tc.tile_pool(name="const", bufs=1))
    qpool = ctx.enter_context(tc.tile_pool(name="qt", bufs=3))
    vbfpool = ctx.enter_context(tc.tile_pool(name="vbf", bufs=8))
    mpool = ctx.enter_context(tc.tile_pool(name="mch", bufs=3))
    tpool = ctx.enter_context(tc.tile_pool(name="tch", bufs=3))
    gates = ctx.enter_context(tc.tile_pool(name="gates", bufs=8))
    small = ctx.enter_context(tc.tile_pool(name="small", bufs=3))
    zlpool = ctx.enter_context(tc.tile_pool(name="zl", bufs=2))
    sigpool = ctx.enter_context(tc.tile_pool(name="sig", bufs=3))
    gchpool = ctx.enter_context(tc.tile_pool(name="gch", bufs=8))
    osbpool = ctx.enter_context(tc.tile_pool(name="osb", bufs=4))

    ps_z = ctx.enter_context(tc.tile_pool(name="psz", bufs=2, space="PSUM"))
    ps_att = ctx.enter_context(tc.tile_pool(name="psatt", bufs=2, space="PSUM"))
    ps_bc = ctx.enter_context(tc.tile_pool(name="psbc", bufs=2, space="PSUM"))
    ps_proj = ctx.enter_context(tc.tile_pool(name="psproj", bufs=2, space="PSUM"))

    # ---------------- const loads ----------------
    cbt = const.tile([128, CB_COLS], BF16)
    nc.gpsimd.dma_start(cbt[:], cb_d[:])            # casting load on Pool q
    cb2t = const.tile([8, C2_COLS], BF16)
    nc.gpsimd.dma_start(cb2t[:], cb2_d[:])
    ones2b = const.tile([128, 2], BF16)
    nc.gpsimd.dma_start(ones2b[:], ones2_d[:])
    ind2b = const.tile([2, 128], BF16)
    nc.gpsimd.dma_start(ind2b[:], ind2_d[:])
    cft = const.tile([128, CF_COLS], F32)
    nc.sync.dma_start(cft[:], cf_d[:])

    owT = cbt[:, CB_OWT:CB_OWT + 2048]
    W3b = cbt[:, CB_W3B:CB_W3B + 577]
    c1blkT = [cbt[:, CB_BLK:CB_BLK + 128], cbt[:, CB_BLK + 128:CB_BLK + 256]]
    dwblkT = [cbt[:, CB_BLK + 256:CB_BLK + 384], cbt[:, CB_BLK + 384:CB_BLK + 512]]
    selT = cbt[:, CB_SEL:CB_SEL + 72]
    ind8 = [cb2t[:, C2_IND8 + 128 * j:C2_IND8 + 128 * (j + 1)] for j in range(4)]
    b1t = cft[:, CF_B1:CF_B1 + 1]
    dbt = cft[:, CF_DB:CF_DB + 1]
    gnw2 = cft[:, CF_GNW:CF_GNW + 1]
    gnb2 = cft[:, CF_GNB:CF_GNB + 1]
    ob4 = cft[:, CF_OB:CF_OB + 4]
    epst = const.tile([128, 1], F32)
    nc.vector.memset(epst[:], EPS)

    # zero-padded z scratch, one per batch; borders + unused quadrant rows
    # stay zero forever, interior rewritten each batch.
    zbigs = []
    for b in range(B_LOC):
        zb = const.tile([128, PS], BF16, name=f"zbig{b}", tag=f"zbig{b}")
        nc.vector.memset(zb[:], 0.0)
        zbigs.append(zb)

    # per-batch state produced by phase A, consumed by phase B
    stA = []

    # ================= phase A: loads, gates, stats, a1, zl =================
    for b in range(B_LOC):
        # q loads (bf16 cast) + k diagonals
        qt = []
        for jp in range(2):
            qb = qpool.tile([128, S], BF16, tag="qt")
            nc.gpsimd.dma_start(
                qb[:], q_d[b, 128 * jp:128 * (jp + 1)].rearrange("c h w -> c (h w)"))
            qt.append(qb)
        kd = []
        for jp in range(2):
            ksl = k_d[b, 128 * jp:128 * (jp + 1)].rearrange("c h w -> c (h w)")
            kt = small.tile([128, 128], F32, tag="kd")
            nc.scalar.dma_start(kt[:, 0:64], ksl[:, 0:4096:65])
            nc.scalar.dma_start(kt[:, 64:128], ksl[:, 63:4095:63])
            kd.append(kt)
        # v loads (bf16 cast)
        vbf = []
        for j in range(4):
            vb = vbfpool.tile([128, S], BF16, tag="vbf")
            nc.gpsimd.dma_start(
                vb[:], v_d[b, 128 * j:128 * (j + 1)].rearrange("c h w -> c (h w)"))
            vbf.append(vb)

        # coordinate + diag gates per v-pack
        hwg = []   # [128,128] bf16: cols 0:64 h-gate(y), 64:128 w-gate(x)
        gd = []    # [128,1] bf16 diag gate
        for jp in range(2):
            hwf = small.tile([128, 128], BF16, tag="hwf")
            with nc.allow_low_precision(reason="bf16 gate logits"):
                nc.vector.reduce_sum(hwf[:, 0:64],
                                     qt[jp][:].rearrange("c (h w) -> c h w", h=H),
                                     axis=AX.X)
                nc.vector.reduce_sum(hwf[:, 64:128],
                                     qt[jp][:].rearrange("c (h w) -> c w h", h=H),
                                     axis=AX.X)
            kds = small.tile([128, 1], BF16, tag="kds")
            with nc.allow_low_precision(reason="bf16 gate logits"):
                nc.vector.reduce_sum(kds[:], kd[jp][:], axis=AX.X)
            for mg in range(2):
                hw_ps = ps_bc.tile([128, 512], F32, tag="bc")
                nc.tensor.matmul(hw_ps[:, 0:128], c1blkT[mg], hwf[:],
                                 start=True, stop=True)
                hg = gates.tile([128, 128], BF16, tag="hwg")
                nc.scalar.activation(hg[:], hw_ps[:, 0:128], AF.Sigmoid,
                                     bias=b1t, scale=1.0 / 64)
                hwg.append(hg)

                g_ps = ps_att.tile([128, 512], F32, tag="att")
                nc.tensor.matmul(g_ps[:, 0:1], dwblkT[mg], kds[:],
                                 start=True, stop=True)
                gt = gates.tile([128, 1], BF16, tag="gd")
                nc.scalar.activation(gt[:], g_ps[:, 0:1], AF.Sigmoid,
                                     bias=dbt, scale=1.0 / 64)
                gd.append(gt)

        # ---- stats over x1 = v*(hg x wg + gd): per-chunk accumulation ----
        statp = small.tile([128, 8], F32, tag="statp")   # cols 2j: pmu, 2j+1: pex2
        for j in range(4):
            s1p = small.tile([128, NCH], F32, tag="s1p")
            s2p = small.tile([128, NCH], F32, tag="s2p")
            for ch in range(NCH):
                sl = slice(CH * ch, CH * (ch + 1))
                mch = mpool.tile([128, CH], BF16)
                hv = hwg[j][:, 8 * ch:8 * (ch + 1)].unsqueeze(2).broadcast_to([128, 8, 64])
                wv = hwg[j][:, 64:128].unsqueeze(1).broadcast_to([128, 8, 64])
                nc.gpsimd.tensor_tensor(
                    mch[:].rearrange("c (h w) -> c h w", h=8), hv, wv, op=ALU.mult)
                tch = tpool.tile([128, CH], BF16)
                with nc.allow_low_precision(reason="bf16 stats staging"):
                    nc.vector.scalar_tensor_tensor(
                        tch[:], mch[:], gd[j][:], vbf[j][:, sl],
                        op0=ALU.add, op1=ALU.mult, accum_out=s1p[:, ch:ch + 1])
                    sq = tpool.tile([128, CH], BF16, tag="sq")
                    nc.vector.scalar_tensor_tensor(
                        sq[:], tch[:], 1.0, tch[:],
                        op0=ALU.mult, op1=ALU.mult, accum_out=s2p[:, ch:ch + 1])
            s1s = small.tile([128, 1], F32, tag="s1s")
            nc.vector.reduce_sum(s1s[:], s1p[:], axis=AX.X)
            nc.vector.tensor_scalar_mul(statp[:, 2 * j:2 * j + 1], s1s[:], 1.0 / S)
            s2s = small.tile([128, 1], F32, tag="s2s")
            nc.vector.reduce_sum(s2s[:], s2p[:], axis=AX.X)
            nc.vector.tensor_scalar_mul(statp[:, 2 * j + 1:2 * j + 2], s2s[:], 1.0 / S)

        # ---- batched group stats for all 4 packs -> a1 (weffL) ----
        statb = small.tile([128, 8], BF16, tag="statb")
        with nc.allow_low_precision(reason="bf16 group stats"):
            nc.vector.tensor_copy(statb[:], statp[:])
        stat_ps = ps_bc.tile([128, 512], F32, tag="bc")
        nc.tensor.matmul(stat_ps[0:2, 0:8], ones2b[:], statb[:],
                         start=True, stop=True)
        stats_sb = small.tile([2, 8], BF16, tag="statsb")
        with nc.allow_low_precision(reason="bf16 group stats"):
            nc.vector.tensor_copy(stats_sb[:], stat_ps[0:2, 0:8])
        grp_ps = ps_bc.tile([128, 512], F32, tag="bc")
        nc.tensor.matmul(grp_ps[:, 0:8], ind2b[:], stats_sb[:],
                         start=True, stop=True)
        # cols 2j = 64*mu_j, 2j+1 = 64*ex2_j
        muv = small.tile([128, 4], F32, tag="muv")
        nc.vector.tensor_scalar_mul(muv[:], grp_ps[:, 0:8:2], 1.0 / 64)
        ex2v = small.tile([128, 4], F32, tag="ex2v")
        nc.vector.tensor_scalar_mul(ex2v[:], grp_ps[:, 1:8:2], 1.0 / 64)
        sqmu = small.tile([128, 4], F32, tag="sqmu")
        nc.vector.tensor_mul(sqmu[:], muv[:], muv[:])
        negvar = small.tile([128, 4], F32, tag="negvar")
        nc.vector.tensor_sub(negvar[:], sqmu[:], ex2v[:])
        stdv = small.tile([128, 4], F32, tag="stdv")
        nc.scalar.activation(stdv[:], negvar[:], AF.Sqrt, bias=epst[:], scale=-1.0)
        inv = small.tile([128, 4], F32, tag="inv")
        nc.vector.reciprocal(inv[:], stdv[:])
        dmu = small.tile([128, 4], F32, tag="dmu")
        nc.vector.tensor_sub(dmu[:], statp[:, 0:8:2], muv[:])
        dsc = small.tile([128, 4], F32, tag="dsc")
        nc.vector.tensor_mul(dsc[:], dmu[:], inv[:])
        logit = small.tile([128, 4], F32, tag="logit")
        nc.vector.scalar_tensor_tensor(
            logit[:], dsc[:], gnw2, gnb2.broadcast_to([128, 4]),
            op0=ALU.mult, op1=ALU.add)
        etb = small.tile([128, 4], BF16, tag="etb")
        nc.scalar.activation(etb[:], logit[:], AF.Exp)
        den_ps = ps_bc.tile([128, 512], F32, tag="bc")
        nc.tensor.matmul(den_ps[0:2, 0:4], ones2b[:], etb[:], start=True, stop=True)
        rden = small.tile([2, 4], F32, tag="rden")
        nc.vector.reciprocal(rden[:], den_ps[0:2, 0:4])
        rdenb = small.tile([2, 4], BF16, tag="rdenb")
        with nc.allow_low_precision(reason="bf16 softmax"):
            nc.vector.tensor_copy(rdenb[:], rden[:])
        rd_ps = ps_bc.tile([128, 512], F32, tag="bc")
        nc.tensor.matmul(rd_ps[:, 0:4], ind2b[:], rdenb[:], start=True, stop=True)
        weffL = small.tile([128, 8], BF16, tag="weffL")
        nc.vector.memset(weffL[:], 0.0)
        with nc.allow_low_precision(reason="bf16 a1"):
            nc.vector.tensor_mul(weffL[0:64, 0:8:2], etb[0:64, :], rd_ps[0:64, 0:4])
            nc.vector.tensor_mul(weffL[64:128, 1:8:2], etb[64:128, :], rd_ps[64:128, 0:4])

        # ---- w_eff = a1 . conv3_w (+bias), then scatter into z lhsT ----
        wex_ps = ps_att.tile([8, 512], F32, tag="att")
        nc.tensor.matmul(wex_ps[:, 0:1], weffL[:], W3b[:, 576:577],
                         start=True, stop=True)
        btt = small.tile([8, 1], F32, tag="btt")
        nc.vector.tensor_copy(btt[:], wex_ps[:, 0:1])

        # transpose w_eff on the PE: tap t matmul (W3b tap-slice)^T @ weffL
        # gives wT[ci, bg] at psum cols 8t; then zl[64b + ci, 32j + 2t + b]
        # = weff[2j+b][64t + ci] via 8 strided copies.
        wt_ps = ps_z.tile([128, 512], F32, tag="z")
        for t9 in range(9):
            nc.tensor.matmul(wt_ps[0:64, 8 * t9:8 * t9 + 8],
                             W3b[:, 64 * t9:64 * t9 + 64], weffL[:],
                             start=True, stop=True)
        zl = zlpool.tile([128, 128], BF16)
        nc.vector.memset(zl[:], 0.0)
        for j in range(4):
            for bb in range(2):
                dst = zl[64 * bb:64 * bb + 64,
                         32 * j + bb:32 * j + bb + 18:2]          # [64, 9]
                src = wt_ps[0:64, 2 * j + bb:2 * j + bb + 65:8]   # [64, 9]
                with nc.allow_low_precision(reason="bf16 z lhsT"):
                    nc.vector.tensor_copy(dst, src)

        stA.append(dict(vbf=vbf, hwg=hwg, gd=gd, zl=zl, btt=btt))

    # ================= phase B: z, att, gate, project =================
    for b in range(B_LOC):
        A = stA[b]
        vbf, zl, btt = A["vbf"], A["zl"], A["btt"]
        zbig = zbigs[b]
        zview = zbig[:].rearrange("p (yy xx) -> p yy xx", yy=PW)

        def emit_z(sc):
            zp = ps_z.tile([128, 512], F32, tag="z")
            for j in range(4):
                nc.tensor.matmul(zp[32 * j:32 * j + 32, :],
                                 zl[:, 32 * j:32 * j + 32],
                                 vbf[j][:, CH * sc:CH * (sc + 1)],
                                 start=True, stop=True,
                                 tile_position=(0, 32 * j))
            with nc.allow_low_precision(reason="bf16 z"):
                nc.scalar.copy(zview[:, 8 * sc + 1:8 * sc + 9, 1:65], zp[:])

        emit_z(0)
        emit_z(1)
        for sc in range(NCH):
            if sc + 2 < NCH:
                emit_z(sc + 2)
            # 9-tap shifted accumulation
            att_ps = ps_att.tile([8, 512], F32, tag="att")
            for t9 in range(9):
                dy, dx = divmod(t9, 3)
                rhs = zview[:, 8 * sc + dy:8 * sc + dy + 8, dx:dx + 64]
                nc.tensor.matmul(att_ps[:], selT[:, 8 * t9:8 * t9 + 8], rhs,
                                 start=(t9 == 0), stop=(t9 == 8))
            sig = sigpool.tile([8, CH], BF16)
            nc.scalar.activation(sig[:], att_ps[:], AF.Sigmoid,
                                 bias=btt[:], scale=1.0)
            # broadcast sigma to 128 partitions per pack, gate v
            gch = []
            for j in range(4):
                bc_ps = ps_bc.tile([128, 512], F32, tag="bc")
                nc.tensor.matmul(bc_ps[:], ind8[j], sig[:], start=True, stop=True)
                g = gchpool.tile([128, CH], BF16, tag="gch")
                with nc.allow_low_precision(reason="bf16 gated v"):
                    nc.vector.tensor_mul(g[:], vbf[j][:, CH * sc:CH * (sc + 1)],
                                         bc_ps[:])
                gch.append(g)
            # output projection
            for oc in range(4):
                pps = ps_proj.tile([128, 512], F32, tag="proj")
                for j in range(4):
                    nc.tensor.matmul(
                        pps[:], owT[:, 512 * j + 128 * oc:512 * j + 128 * oc + 128],
                        gch[j][:], start=(j == 0), stop=(j == 3))
                osb = osbpool.tile([128, CH], F32)
                nc.scalar.activation(osb[:], pps[:], AF.Identity,
                                     bias=ob4[:, oc:oc + 1], scale=1.0)
                nc.sync.dma_start(
                    out_d[b, 128 * oc:128 * (oc + 1)].rearrange("c h w -> c (h w)")[
                        :, CH * sc:CH * (sc + 1)], osb[:])


_NC_CACHE = None


def _get_nc():
    global _NC_CACHE
    if _NC_CACHE is None:
        _NC_CACHE = build_program()
    return _NC_CACHE


def make_core_inputs(inputs, core):
    """Per-core in_map: batch shard + params + host-precomputed consts."""
    params = {n: np.asarray(inputs[n], np.float32) for n in PARAM_NAMES}
    cb, cb2, cf, ones2, ind2 = _host_consts(params)
    sl = slice(B_LOC * core, B_LOC * (core + 1))
    return {
        "q": np.ascontiguousarray(np.asarray(inputs["q"], np.float32)[sl]),
        "k": np.ascontiguousarray(np.asarray(inputs["k"], np.float32)[sl]),
        "v": np.ascontiguousarray(np.asarray(inputs["v"], np.float32)[sl]),
        "cb": cb, "cb2": cb2, "cf": cf, "ones2": ones2, "ind2": ind2,
    }


def kernel(**inputs):
    from concourse.bass_utils import run_bass_kernel_spmd

    nc = _get_nc()
    core_ids = list(range(N_CORES))
    in_maps = [make_core_inputs(inputs, c) for c in core_ids]
    res = run_bass_kernel_spmd(nc, in_maps, core_ids)
    out = np.concatenate([res.results[c]["out"] for c in core_ids], axis=0)
    return out
